# revision 24
# baseline (speedup 1.0000x reference)
"""PointNet FP module (3-NN inverse-distance interpolation + 2-layer pointwise MLP)
as a Bass/Tile kernel for Trainium2, data-parallel over batch across 8 NeuronCores.

Contract: kernel(**inputs) takes the FULL unsharded inputs (numpy arrays, keyed as in
setup_inputs()) and returns the FULL output tuple (out, points_coords, condition).
"""

import sys

sys.path.insert(0, "/opt/trn_rl_repo")

from contextlib import ExitStack

import numpy as np

import concourse.bass as bass
import concourse.tile as tile
from concourse import bacc, mybir
from concourse import bass_utils

F32 = mybir.dt.float32
I16 = mybir.dt.int16
U16 = mybir.dt.uint16

# Problem dims (hardcoded per spec)
B, N, M = 16, 4096, 1024
CC, CP = 256, 64
CIN, H1, H2 = 320, 256, 128
NCORES = 8
B2 = B // NCORES  # batches per core
NT = N // 128  # point tiles per batch (32)
EPS = 1e-10

# Gather chunking: slots are (tile, k) t-major: slot sigma = t*3 + k, 96 slots/batch.
# 4 gather calls per batch, 8 tiles (24 slots, 3072 rows) each.
GCHUNKS = 4
TILES_PER_CHUNK = NT // GCHUNKS  # 8
SLOTS_PER_CHUNK = TILES_PER_CHUNK * 3  # 24
IDX_PER_CHUNK = SLOTS_PER_CHUNK * 128  # 3072

_COMPILED = None  # cache (nc, tensor names) across calls


def _perm_matrix() -> np.ndarray:
    """Pi[p, j] = 1 where j = (p % 16) * 8 + p // 16  (so MT2 col q*8+g = point row 16g+q)."""
    P = np.zeros((128, 128), dtype=np.float32)
    for p in range(128):
        P[p, (p % 16) * 8 + p // 16] = 1.0
    return P


def build_kernel():
    nc = bacc.Bacc("TRN2", target_bir_lowering=False, debug=False, enable_asserts=False)

    # ---- DRAM I/O (per core) ----
    pts_d = nc.dram_tensor("pts", [B2, 3, N], F32, kind="ExternalInput")
    ctr_d = nc.dram_tensor("ctr", [B2, 3, M], F32, kind="ExternalInput")
    ftT_d = nc.dram_tensor("ftT", [B2, M, CC], F32, kind="ExternalInput")
    pf_d = nc.dram_tensor("pf", [B2, CP, N], F32, kind="ExternalInput")
    w1t_d = nc.dram_tensor("w1t", [CIN, H1], F32, kind="ExternalInput")
    w2t_d = nc.dram_tensor("w2t", [H1, H2], F32, kind="ExternalInput")
    b1_d = nc.dram_tensor("b1r", [128, 2], F32, kind="ExternalInput")
    b2_d = nc.dram_tensor("b2r", [128, 1], F32, kind="ExternalInput")
    iden_d = nc.dram_tensor("iden", [128, 128], F32, kind="ExternalInput")
    ones_d = nc.dram_tensor("ones", [1, N], F32, kind="ExternalInput")
    out_d = nc.dram_tensor("out", [B2, H2, N], F32, kind="ExternalOutput")

    with tile.TileContext(nc) as tc, ExitStack() as ctx:
        # ---- pools ----
        const_p = ctx.enter_context(tc.tile_pool(name="const", bufs=1))
        batch_p = ctx.enter_context(tc.tile_pool(name="batch", bufs=B2))
        big_p = ctx.enter_context(tc.tile_pool(name="big", bufs=1))
        gath_p = ctx.enter_context(tc.tile_pool(name="gath", bufs=8))
        work_p = ctx.enter_context(tc.tile_pool(name="work", bufs=2))
        mlpout_p = ctx.enter_context(tc.tile_pool(name="mlpout", bufs=3))
        s_ps = ctx.enter_context(tc.tile_pool(name="s_ps", bufs=1, space="PSUM"))
        oht_ps = ctx.enter_context(tc.tile_pool(name="oht_ps", bufs=1, space="PSUM"))
        tr_ps = ctx.enter_context(tc.tile_pool(name="tr_ps", bufs=1, space="PSUM"))
        mlp_ps = ctx.enter_context(tc.tile_pool(name="mlp_ps", bufs=1, space="PSUM"))

        # ---- constants ----
        w1t_sb = const_p.tile([128, 3, H1], F32)  # K-chunks of W1^T; chunk2 rows 0:64
        nc.sync.dma_start(w1t_sb[:, 0, :], w1t_d.ap()[0:128, :])
        nc.sync.dma_start(w1t_sb[:, 1, :], w1t_d.ap()[128:256, :])
        nc.sync.dma_start(w1t_sb[0:64, 2, :], w1t_d.ap()[256:320, :])
        w2t_sb = const_p.tile([128, 2, H2], F32)
        nc.sync.dma_start(w2t_sb[:, 0, :], w2t_d.ap()[0:128, :])
        nc.sync.dma_start(w2t_sb[:, 1, :], w2t_d.ap()[128:256, :])
        b1_sb = const_p.tile([128, 2], F32)
        nc.sync.dma_start(b1_sb[:], b1_d.ap())
        b2_sb = const_p.tile([128, 1], F32)
        nc.sync.dma_start(b2_sb[:], b2_d.ap())
        iden_sb = const_p.tile([128, 128], F32)
        nc.sync.dma_start(iden_sb[:], iden_d.ap())
        ones1_sb = const_p.tile([128, 1], F32)
        nc.vector.memset(ones1_sb[:], 1.0)

        for b in range(B2):
            # ---- per-batch loads / prep ----
            pts4 = big_p.tile([4, N], F32, tag="pts4")
            nc.sync.dma_start(pts4[0:3, :], pts_d.ap()[b])
            nc.sync.dma_start(pts4[3:4, :], ones_d.ap())

            pxyz = batch_p.tile([32, 3, 128], F32, tag="pxyz")
            for d in range(3):
                nc.sync.dma_start(
                    pxyz[:, d, :],
                    pts_d.ap()[b, d].rearrange("(t p) -> t p", p=128),
                )

            ctr_sb = batch_p.tile([3, M], F32, tag="ctr")
            nc.sync.dma_start(ctr_sb[:], ctr_d.ap()[b])
            ctr1 = batch_p.tile([1, 3, M], F32, tag="ctr1")
            nc.sync.dma_start(ctr1[:], ctr_d.ap()[b].unsqueeze(0))

            # rhs4 = [2*cx; 2*cy; 2*cz; -c2]  (so s = lhsT^T@rhs4 = 2<p,c> - c2;
            # top-8 max of s = 8 smallest d2 since d2 = p2 - s)
            rhs4 = batch_p.tile([4, M], F32, tag="rhs4")
            nc.scalar.activation(
                rhs4[0:3, :], ctr_sb[:], mybir.ActivationFunctionType.Copy, scale=2.0
            )
            # c2 on a single partition (DVE can't address base partitions 1/2),
            # then DMA the negated row into rhs4[3]
            nc.scalar.activation(ctr1[:], ctr1[:], mybir.ActivationFunctionType.Square)
            c2n = batch_p.tile([1, M], F32, tag="c2n")
            nc.vector.tensor_tensor(
                c2n[:], ctr1[:, 0, :], ctr1[:, 1, :], op=mybir.AluOpType.add
            )
            nc.vector.tensor_tensor(
                c2n[:], c2n[:], ctr1[:, 2, :], op=mybir.AluOpType.add
            )
            nc.scalar.activation(
                c2n[:], c2n[:], mybir.ActivationFunctionType.Copy, scale=-1.0
            )
            nc.sync.dma_start(rhs4[3:4, :], c2n[:])

            # p2 per point, laid out [128, NT] (column t = tile t's points)
            nc.scalar.activation(pxyz[:], pxyz[:], mybir.ActivationFunctionType.Square)
            p2a = batch_p.tile([32, 128], F32, tag="p2a")
            nc.vector.tensor_tensor(
                p2a[:], pxyz[:, 0, :], pxyz[:, 1, :], op=mybir.AluOpType.add
            )
            p2b = batch_p.tile([32, 128], F32, tag="p2b")
            nc.vector.tensor_tensor(
                p2b[:], p2a[:], pxyz[:, 2, :], op=mybir.AluOpType.add
            )
            p2T = batch_p.tile([128, 32], F32, tag="p2T")
            for j in range(4):
                nc.vector.transpose(
                    p2T[32 * j : 32 * (j + 1), 0:32], p2b[0:32, 32 * j : 32 * (j + 1)]
                )

            # ---- per-tile pipeline in groups of 8: distances -> top3 values ->
            #      weighted one-hot masks -> PE transpose (accumulating over k) ->
            #      interp = sum_chunks FT_chunk^T @ OHT_chunk ----
            ft_sb = big_p.tile([128, 8, CC], F32, tag="ft_sb")
            nc.sync.dma_start(
                ft_sb[:], ftT_d.ap()[b].rearrange("(c p) f -> p c f", p=128)
            )
            itc = big_p.tile([128, 2, N], F32, tag="itc")  # interp, C-halves

            t8 = batch_p.tile([128, NT, 8], F32, tag="t8")
            G = 8
            for g in range(NT // G):
                sS_tiles = []
                for tt in range(G):
                    t = g * G + tt
                    sps = s_ps.tile([128, M], F32, tag="s", name=f"sps{b}_{t}")
                    lhsT = pts4[:, 128 * t : 128 * (t + 1)]
                    nc.tensor.matmul(sps[:, 0:512], lhsT, rhs4[:, 0:512], start=True, stop=True)
                    nc.tensor.matmul(sps[:, 512:1024], lhsT, rhs4[:, 512:1024], start=True, stop=True)
                    sS = gath_p.tile([128, M], F32, tag="sS", name=f"sS{b}_{t}")
                    nc.scalar.activation(sS[:], sps[:], mybir.ActivationFunctionType.Copy)
                    nc.vector.max(t8[:, t, :], sS[:])
                    sS_tiles.append(sS)

                # weights for this group: w_k = (1/d2_k)/sum(1/d2_k), d2_k = p2 - t_k
                tsl = slice(g * G, (g + 1) * G)
                d3 = batch_p.tile([128, G, 3], F32, tag="d3", name=f"d3{b}_{g}")
                p2bc = p2T[:, tsl].unsqueeze(-1).broadcast_to((128, G, 3))
                nc.vector.tensor_tensor(
                    d3[:], p2bc, t8[:, tsl, 0:3], op=mybir.AluOpType.subtract
                )
                nc.vector.tensor_scalar(d3[:], d3[:], EPS, None, op0=mybir.AluOpType.max)
                r3 = batch_p.tile([128, G, 3], F32, tag="r3", name=f"r3{b}_{g}")
                nc.vector.reciprocal(r3[:], d3[:])
                rs = batch_p.tile([128, G], F32, tag="rs", name=f"rs{b}_{g}")
                nc.vector.tensor_reduce(
                    rs[:], r3[:], axis=mybir.AxisListType.X, op=mybir.AluOpType.add
                )
                nc.vector.reciprocal(rs[:], rs[:])
                # telescoped coefficients: a_k = (r_k - r_{k+1})/sum_r (r4 := 0)
                rd = batch_p.tile([128, G, 3], F32, tag="rd", name=f"rd{b}_{g}")
                nc.vector.tensor_copy(rd[:, :, 2:3], r3[:, :, 2:3])
                nc.vector.tensor_tensor(
                    rd[:, :, 0:2], r3[:, :, 0:2], r3[:, :, 1:3],
                    op=mybir.AluOpType.subtract,
                )
                w3 = batch_p.tile([128, G, 3], F32, tag="w3", name=f"w3{b}_{g}")
                rsbc = rs[:].unsqueeze(-1).broadcast_to((128, G, 3))
                nc.vector.tensor_tensor(w3[:], rd[:], rsbc, op=mybir.AluOpType.mult)

                for tt in range(G):
                    t = g * G + tt
                    sS = sS_tiles[tt]
                    # weighted one-hot masks, one fused TSP per neighbor k
                    tks = []
                    for k in range(3):
                        tk = work_p.tile([128, M], F32, tag=f"tk{k}", name=f"tk{b}_{t}_{k}")
                        nc.vector.tensor_scalar(
                            tk[:], sS[:], t8[:, t, k + 1 : k + 2], w3[:, tt, k : k + 1],
                            op0=mybir.AluOpType.is_gt, op1=mybir.AluOpType.mult,
                        )
                        tks.append(tk)
                    # OHT[mchunk][m, p] = sum_k Tk[p, m]: accumulate transposes in PSUM
                    oht0 = oht_ps.tile([128, 512], F32, tag="oht0", name=f"oht0_{b}_{t}")
                    oht1 = oht_ps.tile([128, 512], F32, tag="oht1", name=f"oht1_{b}_{t}")
                    for c in range(8):
                        dst = (oht0 if c < 4 else oht1)[:, 128 * (c % 4) : 128 * (c % 4 + 1)]
                        for k in range(3):
                            nc.tensor.matmul(
                                dst, tks[k][:, 128 * c : 128 * (c + 1)], iden_sb[:],
                                start=(k == 0), stop=(k == 2), is_transpose=True,
                            )
                    ohts = work_p.tile([128, 8, 128], F32, tag="ohts", name=f"ohts{b}_{t}")
                    nc.scalar.activation(
                        ohts[:, 0:4, :], oht0[:], mybir.ActivationFunctionType.Copy
                    )
                    nc.scalar.activation(
                        ohts[:, 4:8, :], oht1[:], mybir.ActivationFunctionType.Copy
                    )
                    # interp accumulation over the 8 M-chunks
                    grp, pos = t // 4, t % 4
                    if pos == 0:
                        trh0 = tr_ps.tile([128, 512], F32, tag="trh0", name=f"trh0_{b}_{t}")
                        trh1 = tr_ps.tile([128, 512], F32, tag="trh1", name=f"trh1_{b}_{t}")
                        cur = (trh0, trh1)
                    trh0, trh1 = cur
                    psl = slice(128 * pos, 128 * (pos + 1))
                    for c in range(8):
                        nc.tensor.matmul(
                            trh0[:, psl], ft_sb[:, c, 0:128], ohts[:, c, :],
                            start=(c == 0), stop=(c == 7),
                        )
                        nc.tensor.matmul(
                            trh1[:, psl], ft_sb[:, c, 128:256], ohts[:, c, :],
                            start=(c == 0), stop=(c == 7),
                        )
                    if pos == 3:
                        nc.scalar.activation(
                            itc[:, 0, 512 * grp : 512 * (grp + 1)], trh0[:],
                            mybir.ActivationFunctionType.Copy,
                        )
                        nc.scalar.activation(
                            itc[:, 1, 512 * grp : 512 * (grp + 1)], trh1[:],
                            mybir.ActivationFunctionType.Copy,
                        )

            # ---- MLP ----
            for nt in range(N // 512):
                sl = slice(512 * nt, 512 * (nt + 1))
                hps = [
                    mlp_ps.tile([128, 512], F32, tag=f"h{blk}", name=f"hps{blk}")
                    for blk in range(2)
                ]
                pf_sb = mlpout_p.tile([64, 512], F32, tag="pfs", name=f"pfs{b}_{nt}")
                nc.sync.dma_start(pf_sb[:], pf_d.ap()[b, :, sl])
                for blk in range(2):
                    cb = slice(128 * blk, 128 * (blk + 1))
                    nc.tensor.matmul(
                        hps[blk][:], w1t_sb[:, 0, cb], itc[:, 0, sl], start=True, stop=False
                    )
                    nc.tensor.matmul(
                        hps[blk][:], w1t_sb[:, 1, cb], itc[:, 1, sl], start=False, stop=False
                    )
                    nc.tensor.matmul(
                        hps[blk][:], w1t_sb[0:64, 2, cb], pf_sb[:], start=False, stop=True
                    )
                hsb = work_p.tile([128, 2, 512], F32, tag="hsb")
                for blk in range(2):
                    nc.scalar.activation(
                        hsb[:, blk, :], hps[blk][:],
                        mybir.ActivationFunctionType.Relu, bias=b1_sb[:, blk : blk + 1],
                    )
                ops = mlp_ps.tile([128, 512], F32, tag="h0")
                nc.tensor.matmul(ops[:], w2t_sb[:, 0, :], hsb[:, 0, :], start=True, stop=False)
                nc.tensor.matmul(ops[:], w2t_sb[:, 1, :], hsb[:, 1, :], start=False, stop=True)
                osb = mlpout_p.tile([128, 512], F32, tag="osb")
                nc.scalar.activation(
                    osb[:], ops[:], mybir.ActivationFunctionType.Relu, bias=b2_sb[:]
                )
                nc.sync.dma_start(out_d.ap()[b, :, sl], osb[:])

    nc.compile()
    return nc


def kernel(points_coords, centers_coords, centers_features, points_features,
           condition, W1, b1, W2, b2):
    global _COMPILED
    points_coords = np.ascontiguousarray(np.asarray(points_coords, dtype=np.float32))
    centers_coords = np.ascontiguousarray(np.asarray(centers_coords, dtype=np.float32))
    centers_features = np.asarray(centers_features, dtype=np.float32)
    points_features = np.ascontiguousarray(np.asarray(points_features, dtype=np.float32))
    W1 = np.asarray(W1, dtype=np.float32)
    W2 = np.asarray(W2, dtype=np.float32)
    b1 = np.asarray(b1, dtype=np.float32)
    b2 = np.asarray(b2, dtype=np.float32)

    if _COMPILED is None:
        _COMPILED = build_kernel()
    nc = _COMPILED

    ftT = np.ascontiguousarray(centers_features.transpose(0, 2, 1))  # (B, M, CC)
    w1t = np.ascontiguousarray(W1.T)  # (320, 256)
    w2t = np.ascontiguousarray(W2.T)  # (256, 128)
    b1r = np.ascontiguousarray(b1.reshape(2, 128).T)  # (128, 2)
    b2r = np.ascontiguousarray(b2.reshape(1, 128).T)  # (128, 1)
    iden = np.eye(128, dtype=np.float32)
    ones = np.ones((1, N), dtype=np.float32)

    in_maps = []
    for c in range(NCORES):
        s = slice(B2 * c, B2 * (c + 1))
        in_maps.append({
            "pts": points_coords[s],
            "ctr": centers_coords[s],
            "ftT": ftT[s],
            "pf": points_features[s],
            "w1t": w1t,
            "w2t": w2t,
            "b1r": b1r,
            "b2r": b2r,
            "iden": iden,
            "ones": ones,
        })

    res = bass_utils.run_bass_kernel_spmd(nc, in_maps, core_ids=list(range(NCORES)))
    out = np.concatenate([res.results[c]["out"] for c in range(NCORES)], axis=0)
    return (
        out.astype(np.float32),
        points_coords,
        np.asarray(condition, dtype=np.float32),
    )


if __name__ == "__main__":
    rng = np.random.default_rng(0)
    ins = {
        "points_coords": rng.random((B, 3, N), dtype=np.float32),
        "centers_coords": rng.random((B, 3, M), dtype=np.float32),
        "centers_features": rng.standard_normal((B, CC, M), dtype=np.float32),
        "points_features": rng.standard_normal((B, CP, N), dtype=np.float32),
        "condition": rng.standard_normal((B, 128), dtype=np.float32),
        "W1": rng.standard_normal((H1, CIN), dtype=np.float32) / np.sqrt(CIN),
        "b1": np.zeros(H1, dtype=np.float32),
        "W2": rng.standard_normal((H2, H1), dtype=np.float32) / np.sqrt(H1),
        "b2": np.zeros(H2, dtype=np.float32),
    }
    out = kernel(**ins)
    print([o.shape for o in out])


# revision 25
# speedup vs baseline: 1.1835x; 1.1835x over previous
"""PointNet FP module (3-NN inverse-distance interpolation + 2-layer pointwise MLP)
as a Bass/Tile kernel for Trainium2, data-parallel over batch across 8 NeuronCores.

Contract: kernel(**inputs) takes the FULL unsharded inputs (numpy arrays, keyed as in
setup_inputs()) and returns the FULL output tuple (out, points_coords, condition).
"""

import sys

sys.path.insert(0, "/opt/trn_rl_repo")

from contextlib import ExitStack

import numpy as np

import concourse.bass as bass
import concourse.tile as tile
from concourse import bacc, mybir
from concourse import bass_utils

F32 = mybir.dt.float32
I16 = mybir.dt.int16
U16 = mybir.dt.uint16

# Problem dims (hardcoded per spec)
B, N, M = 16, 4096, 1024
CC, CP = 256, 64
CIN, H1, H2 = 320, 256, 128
NCORES = 8
B2 = B // NCORES  # batches per core
NT = N // 128  # point tiles per batch (32)
EPS = 1e-10

# Gather chunking: slots are (tile, k) t-major: slot sigma = t*3 + k, 96 slots/batch.
# 4 gather calls per batch, 8 tiles (24 slots, 3072 rows) each.
GCHUNKS = 4
TILES_PER_CHUNK = NT // GCHUNKS  # 8
SLOTS_PER_CHUNK = TILES_PER_CHUNK * 3  # 24
IDX_PER_CHUNK = SLOTS_PER_CHUNK * 128  # 3072

_COMPILED = None  # cache (nc, tensor names) across calls


def _perm_matrix() -> np.ndarray:
    """Pi[p, j] = 1 where j = (p % 16) * 8 + p // 16  (so MT2 col q*8+g = point row 16g+q)."""
    P = np.zeros((128, 128), dtype=np.float32)
    for p in range(128):
        P[p, (p % 16) * 8 + p // 16] = 1.0
    return P


def build_kernel():
    nc = bacc.Bacc("TRN2", target_bir_lowering=False, debug=False, enable_asserts=False)

    # ---- DRAM I/O (per core) ----
    pts_d = nc.dram_tensor("pts", [B2, 3, N], F32, kind="ExternalInput")
    ctr_d = nc.dram_tensor("ctr", [B2, 3, M], F32, kind="ExternalInput")
    ftT_d = nc.dram_tensor("ftT", [B2, M, CC], F32, kind="ExternalInput")
    pf_d = nc.dram_tensor("pf", [B2, CP, N], F32, kind="ExternalInput")
    w1t_d = nc.dram_tensor("w1t", [CIN, H1], F32, kind="ExternalInput")
    w2t_d = nc.dram_tensor("w2t", [H1, H2], F32, kind="ExternalInput")
    b1_d = nc.dram_tensor("b1r", [128, 2], F32, kind="ExternalInput")
    b2_d = nc.dram_tensor("b2r", [128, 1], F32, kind="ExternalInput")
    iden_d = nc.dram_tensor("iden", [128, 128], F32, kind="ExternalInput")
    ones_d = nc.dram_tensor("ones", [1, N], F32, kind="ExternalInput")
    out_d = nc.dram_tensor("out", [B2, H2, N], F32, kind="ExternalOutput")

    with tile.TileContext(nc) as tc, ExitStack() as ctx:
        # ---- pools ----
        const_p = ctx.enter_context(tc.tile_pool(name="const", bufs=1))
        batch_p = ctx.enter_context(tc.tile_pool(name="batch", bufs=B2))
        big_p = ctx.enter_context(tc.tile_pool(name="big", bufs=1))
        gath_p = ctx.enter_context(tc.tile_pool(name="gath", bufs=8))
        work_p = ctx.enter_context(tc.tile_pool(name="work", bufs=2))
        mlpout_p = ctx.enter_context(tc.tile_pool(name="mlpout", bufs=3))
        s_ps = ctx.enter_context(tc.tile_pool(name="s_ps", bufs=1, space="PSUM"))
        oht_ps = ctx.enter_context(tc.tile_pool(name="oht_ps", bufs=1, space="PSUM"))
        tr_ps = ctx.enter_context(tc.tile_pool(name="tr_ps", bufs=1, space="PSUM"))
        mlp_ps = ctx.enter_context(tc.tile_pool(name="mlp_ps", bufs=1, space="PSUM"))

        # ---- constants ----
        w1t_sb = const_p.tile([128, 3, H1], F32)  # K-chunks of W1^T; chunk2 rows 0:64
        nc.sync.dma_start(w1t_sb[:, 0, :], w1t_d.ap()[0:128, :])
        nc.sync.dma_start(w1t_sb[:, 1, :], w1t_d.ap()[128:256, :])
        nc.sync.dma_start(w1t_sb[0:64, 2, :], w1t_d.ap()[256:320, :])
        w2t_sb = const_p.tile([128, 2, H2], F32)
        nc.sync.dma_start(w2t_sb[:, 0, :], w2t_d.ap()[0:128, :])
        nc.sync.dma_start(w2t_sb[:, 1, :], w2t_d.ap()[128:256, :])
        b1_sb = const_p.tile([128, 2], F32)
        nc.sync.dma_start(b1_sb[:], b1_d.ap())
        b2_sb = const_p.tile([128, 1], F32)
        nc.sync.dma_start(b2_sb[:], b2_d.ap())
        iden_sb = const_p.tile([128, 128], F32)
        nc.sync.dma_start(iden_sb[:], iden_d.ap())
        ones1_sb = const_p.tile([128, 1], F32)
        nc.vector.memset(ones1_sb[:], 1.0)

        for b in range(B2):
            # ---- per-batch loads / prep ----
            pts4 = big_p.tile([4, N], F32, tag="pts4")
            nc.sync.dma_start(pts4[0:3, :], pts_d.ap()[b])
            nc.sync.dma_start(pts4[3:4, :], ones_d.ap())

            pxyz = batch_p.tile([32, 3, 128], F32, tag="pxyz")
            for d in range(3):
                nc.sync.dma_start(
                    pxyz[:, d, :],
                    pts_d.ap()[b, d].rearrange("(t p) -> t p", p=128),
                )

            ctr_sb = batch_p.tile([3, M], F32, tag="ctr")
            nc.sync.dma_start(ctr_sb[:], ctr_d.ap()[b])
            ctr1 = batch_p.tile([1, 3, M], F32, tag="ctr1")
            nc.sync.dma_start(ctr1[:], ctr_d.ap()[b].unsqueeze(0))

            # rhs4 = [2*cx; 2*cy; 2*cz; -c2]  (so s = lhsT^T@rhs4 = 2<p,c> - c2;
            # top-8 max of s = 8 smallest d2 since d2 = p2 - s)
            rhs4 = batch_p.tile([4, M], F32, tag="rhs4")
            nc.scalar.activation(
                rhs4[0:3, :], ctr_sb[:], mybir.ActivationFunctionType.Copy, scale=2.0
            )
            # c2 on a single partition (DVE can't address base partitions 1/2),
            # then DMA the negated row into rhs4[3]
            nc.scalar.activation(ctr1[:], ctr1[:], mybir.ActivationFunctionType.Square)
            c2n = batch_p.tile([1, M], F32, tag="c2n")
            nc.vector.tensor_tensor(
                c2n[:], ctr1[:, 0, :], ctr1[:, 1, :], op=mybir.AluOpType.add
            )
            nc.vector.tensor_tensor(
                c2n[:], c2n[:], ctr1[:, 2, :], op=mybir.AluOpType.add
            )
            nc.scalar.activation(
                c2n[:], c2n[:], mybir.ActivationFunctionType.Copy, scale=-1.0
            )
            nc.sync.dma_start(rhs4[3:4, :], c2n[:])

            # p2 per point, laid out [128, NT] (column t = tile t's points)
            nc.scalar.activation(pxyz[:], pxyz[:], mybir.ActivationFunctionType.Square)
            p2a = batch_p.tile([32, 128], F32, tag="p2a")
            nc.vector.tensor_tensor(
                p2a[:], pxyz[:, 0, :], pxyz[:, 1, :], op=mybir.AluOpType.add
            )
            p2b = batch_p.tile([32, 128], F32, tag="p2b")
            nc.vector.tensor_tensor(
                p2b[:], p2a[:], pxyz[:, 2, :], op=mybir.AluOpType.add
            )
            p2T = batch_p.tile([128, 32], F32, tag="p2T")
            for j in range(4):
                nc.vector.transpose(
                    p2T[32 * j : 32 * (j + 1), 0:32], p2b[0:32, 32 * j : 32 * (j + 1)]
                )

            # ---- per-tile pipeline in groups of 8: distances -> top3 values ->
            #      weighted one-hot masks -> PE transpose (accumulating over k) ->
            #      interp = sum_chunks FT_chunk^T @ OHT_chunk ----
            ft_sb = big_p.tile([128, 8, CC], F32, tag="ft_sb")
            nc.sync.dma_start(
                ft_sb[:], ftT_d.ap()[b].rearrange("(c p) f -> p c f", p=128)
            )
            itc = big_p.tile([128, 2, N], F32, tag="itc")  # interp, C-halves

            t8 = batch_p.tile([128, NT, 8], F32, tag="t8")
            G = 8
            for g in range(NT // G):
                sS_tiles = []
                for tt in range(G):
                    t = g * G + tt
                    sps = s_ps.tile([128, M], F32, tag="s", name=f"sps{b}_{t}")
                    lhsT = pts4[:, 128 * t : 128 * (t + 1)]
                    nc.tensor.matmul(sps[:, 0:512], lhsT, rhs4[:, 0:512], start=True, stop=True)
                    nc.tensor.matmul(sps[:, 512:1024], lhsT, rhs4[:, 512:1024], start=True, stop=True)
                    sS = gath_p.tile([128, M], F32, tag="sS", name=f"sS{b}_{t}")
                    nc.scalar.activation(sS[:], sps[:], mybir.ActivationFunctionType.Copy)
                    nc.vector.max(t8[:, t, :], sS[:])
                    sS_tiles.append(sS)

                # weights for this group: w_k = (1/d2_k)/sum(1/d2_k), d2_k = p2 - t_k
                tsl = slice(g * G, (g + 1) * G)
                d3 = batch_p.tile([128, G, 3], F32, tag="d3", name=f"d3{b}_{g}")
                p2bc = p2T[:, tsl].unsqueeze(-1).broadcast_to((128, G, 3))
                nc.vector.tensor_tensor(
                    d3[:], p2bc, t8[:, tsl, 0:3], op=mybir.AluOpType.subtract
                )
                nc.vector.tensor_scalar(d3[:], d3[:], EPS, None, op0=mybir.AluOpType.max)
                r3 = batch_p.tile([128, G, 3], F32, tag="r3", name=f"r3{b}_{g}")
                nc.vector.reciprocal(r3[:], d3[:])
                rs = batch_p.tile([128, G], F32, tag="rs", name=f"rs{b}_{g}")
                nc.vector.tensor_reduce(
                    rs[:], r3[:], axis=mybir.AxisListType.X, op=mybir.AluOpType.add
                )
                nc.vector.reciprocal(rs[:], rs[:])
                # telescoped coefficients: a_k = (r_k - r_{k+1})/sum_r (r4 := 0)
                rd = batch_p.tile([128, G, 3], F32, tag="rd", name=f"rd{b}_{g}")
                nc.vector.tensor_copy(rd[:, :, 2:3], r3[:, :, 2:3])
                nc.vector.tensor_tensor(
                    rd[:, :, 0:2], r3[:, :, 0:2], r3[:, :, 1:3],
                    op=mybir.AluOpType.subtract,
                )
                w3 = batch_p.tile([128, G, 3], F32, tag="w3", name=f"w3{b}_{g}")
                rsbc = rs[:].unsqueeze(-1).broadcast_to((128, G, 3))
                nc.vector.tensor_tensor(w3[:], rd[:], rsbc, op=mybir.AluOpType.mult)

                for tt in range(G):
                    t = g * G + tt
                    sS = sS_tiles[tt]
                    # weighted one-hot masks, one fused TSP per neighbor k
                    tks = []
                    for k in range(3):
                        tk = work_p.tile([128, M], F32, tag=f"tk{k}", name=f"tk{b}_{t}_{k}")
                        nc.vector.tensor_scalar(
                            tk[:], sS[:], t8[:, t, k + 1 : k + 2], w3[:, tt, k : k + 1],
                            op0=mybir.AluOpType.is_gt, op1=mybir.AluOpType.mult,
                        )
                        tks.append(tk)
                    # sum the 3 masks on DVE (disjoint nonzeros), then one
                    # transpose per M-chunk instead of three
                    nc.vector.tensor_tensor(
                        tks[1][:], tks[0][:], tks[1][:], op=mybir.AluOpType.add
                    )
                    nc.vector.tensor_tensor(
                        tks[2][:], tks[1][:], tks[2][:], op=mybir.AluOpType.add
                    )
                    oht0 = oht_ps.tile([128, 512], F32, tag="oht0", name=f"oht0_{b}_{t}")
                    oht1 = oht_ps.tile([128, 512], F32, tag="oht1", name=f"oht1_{b}_{t}")
                    for c in range(8):
                        dst = (oht0 if c < 4 else oht1)[:, 128 * (c % 4) : 128 * (c % 4 + 1)]
                        nc.tensor.matmul(
                            dst, tks[2][:, 128 * c : 128 * (c + 1)], iden_sb[:],
                            start=True, stop=True, is_transpose=True,
                        )
                    ohts = work_p.tile([128, 8, 128], F32, tag="ohts", name=f"ohts{b}_{t}")
                    nc.scalar.activation(
                        ohts[:, 0:4, :], oht0[:], mybir.ActivationFunctionType.Copy
                    )
                    nc.scalar.activation(
                        ohts[:, 4:8, :], oht1[:], mybir.ActivationFunctionType.Copy
                    )
                    # interp accumulation over the 8 M-chunks
                    grp, pos = t // 4, t % 4
                    if pos == 0:
                        trh0 = tr_ps.tile([128, 512], F32, tag="trh0", name=f"trh0_{b}_{t}")
                        trh1 = tr_ps.tile([128, 512], F32, tag="trh1", name=f"trh1_{b}_{t}")
                        cur = (trh0, trh1)
                    trh0, trh1 = cur
                    psl = slice(128 * pos, 128 * (pos + 1))
                    for c in range(8):
                        nc.tensor.matmul(
                            trh0[:, psl], ft_sb[:, c, 0:128], ohts[:, c, :],
                            start=(c == 0), stop=(c == 7),
                        )
                        nc.tensor.matmul(
                            trh1[:, psl], ft_sb[:, c, 128:256], ohts[:, c, :],
                            start=(c == 0), stop=(c == 7),
                        )
                    if pos == 3:
                        nc.scalar.activation(
                            itc[:, 0, 512 * grp : 512 * (grp + 1)], trh0[:],
                            mybir.ActivationFunctionType.Copy,
                        )
                        nc.scalar.activation(
                            itc[:, 1, 512 * grp : 512 * (grp + 1)], trh1[:],
                            mybir.ActivationFunctionType.Copy,
                        )

            # ---- MLP ----
            for nt in range(N // 512):
                sl = slice(512 * nt, 512 * (nt + 1))
                hps = [
                    mlp_ps.tile([128, 512], F32, tag=f"h{blk}", name=f"hps{blk}")
                    for blk in range(2)
                ]
                pf_sb = mlpout_p.tile([64, 512], F32, tag="pfs", name=f"pfs{b}_{nt}")
                nc.sync.dma_start(pf_sb[:], pf_d.ap()[b, :, sl])
                for blk in range(2):
                    cb = slice(128 * blk, 128 * (blk + 1))
                    nc.tensor.matmul(
                        hps[blk][:], w1t_sb[:, 0, cb], itc[:, 0, sl], start=True, stop=False
                    )
                    nc.tensor.matmul(
                        hps[blk][:], w1t_sb[:, 1, cb], itc[:, 1, sl], start=False, stop=False
                    )
                    nc.tensor.matmul(
                        hps[blk][:], w1t_sb[0:64, 2, cb], pf_sb[:], start=False, stop=True
                    )
                hsb = work_p.tile([128, 2, 512], F32, tag="hsb")
                for blk in range(2):
                    nc.scalar.activation(
                        hsb[:, blk, :], hps[blk][:],
                        mybir.ActivationFunctionType.Relu, bias=b1_sb[:, blk : blk + 1],
                    )
                ops = mlp_ps.tile([128, 512], F32, tag="h0")
                nc.tensor.matmul(ops[:], w2t_sb[:, 0, :], hsb[:, 0, :], start=True, stop=False)
                nc.tensor.matmul(ops[:], w2t_sb[:, 1, :], hsb[:, 1, :], start=False, stop=True)
                osb = mlpout_p.tile([128, 512], F32, tag="osb")
                nc.scalar.activation(
                    osb[:], ops[:], mybir.ActivationFunctionType.Relu, bias=b2_sb[:]
                )
                nc.sync.dma_start(out_d.ap()[b, :, sl], osb[:])

    nc.compile()
    return nc


def kernel(points_coords, centers_coords, centers_features, points_features,
           condition, W1, b1, W2, b2):
    global _COMPILED
    points_coords = np.ascontiguousarray(np.asarray(points_coords, dtype=np.float32))
    centers_coords = np.ascontiguousarray(np.asarray(centers_coords, dtype=np.float32))
    centers_features = np.asarray(centers_features, dtype=np.float32)
    points_features = np.ascontiguousarray(np.asarray(points_features, dtype=np.float32))
    W1 = np.asarray(W1, dtype=np.float32)
    W2 = np.asarray(W2, dtype=np.float32)
    b1 = np.asarray(b1, dtype=np.float32)
    b2 = np.asarray(b2, dtype=np.float32)

    if _COMPILED is None:
        _COMPILED = build_kernel()
    nc = _COMPILED

    ftT = np.ascontiguousarray(centers_features.transpose(0, 2, 1))  # (B, M, CC)
    w1t = np.ascontiguousarray(W1.T)  # (320, 256)
    w2t = np.ascontiguousarray(W2.T)  # (256, 128)
    b1r = np.ascontiguousarray(b1.reshape(2, 128).T)  # (128, 2)
    b2r = np.ascontiguousarray(b2.reshape(1, 128).T)  # (128, 1)
    iden = np.eye(128, dtype=np.float32)
    ones = np.ones((1, N), dtype=np.float32)

    in_maps = []
    for c in range(NCORES):
        s = slice(B2 * c, B2 * (c + 1))
        in_maps.append({
            "pts": points_coords[s],
            "ctr": centers_coords[s],
            "ftT": ftT[s],
            "pf": points_features[s],
            "w1t": w1t,
            "w2t": w2t,
            "b1r": b1r,
            "b2r": b2r,
            "iden": iden,
            "ones": ones,
        })

    res = bass_utils.run_bass_kernel_spmd(nc, in_maps, core_ids=list(range(NCORES)))
    out = np.concatenate([res.results[c]["out"] for c in range(NCORES)], axis=0)
    return (
        out.astype(np.float32),
        points_coords,
        np.asarray(condition, dtype=np.float32),
    )


if __name__ == "__main__":
    rng = np.random.default_rng(0)
    ins = {
        "points_coords": rng.random((B, 3, N), dtype=np.float32),
        "centers_coords": rng.random((B, 3, M), dtype=np.float32),
        "centers_features": rng.standard_normal((B, CC, M), dtype=np.float32),
        "points_features": rng.standard_normal((B, CP, N), dtype=np.float32),
        "condition": rng.standard_normal((B, 128), dtype=np.float32),
        "W1": rng.standard_normal((H1, CIN), dtype=np.float32) / np.sqrt(CIN),
        "b1": np.zeros(H1, dtype=np.float32),
        "W2": rng.standard_normal((H2, H1), dtype=np.float32) / np.sqrt(H1),
        "b2": np.zeros(H2, dtype=np.float32),
    }
    out = kernel(**ins)
    print([o.shape for o in out])
